# revision 1
# baseline (speedup 1.0000x reference)
"""NsNet2 single-step (fc1 + 2x GRU cell + 3x FC) Trainium2 kernel.

Strategy:
  - Pure data parallel: batch B=32768 sharded as 4096 rows per NeuronCore (8 cores).
  - Feature-major ("transposed") layout on chip: activations live as [feat, batch]
    so every matmul's moving operand is already in [K, N] form -> zero on-chip
    transposes. Host transposes inputs/outputs (free; not on HW critical path).
  - bf16 matmuls (full PE rate) with fp32 PSUM accumulation; fp32 biases fused
    into ScalarE activation (sigmoid/tanh) or VectorE tensor_scalar (relu).
  - fc1 is folded into the GRU1 input-gate weights on the host (fc1 is linear and
    f1 is consumed only by GRU1's input matmuls):  (x@Wfc1.T+b) @ Wg.T =
    x @ (Wg@Wfc1).T + (Wg@b + bg).
  - z,r gates sum their input-side and hidden-side matmuls in one PSUM, so their
    contraction operands are K-concatenated ([x|h1] resp. [g1|h2]) on the host /
    on chip, saving ceil() waste: GRU1 zr K=657->6 chunks (vs 3+4), GRU2
    K=800->7 (vs 4+4).
  - Feature dims zero-padded to multiples of 128 where needed; padding never
    increases PE chunk count and keeps matmul contraction at 128 partitions.
"""

import os
import sys

import numpy as np
import ml_dtypes

sys.path.insert(0, "/opt/trn_rl_repo")

import concourse.bacc as bacc
import concourse.bass as bass
import concourse.mybir as mybir
import concourse.tile as tile
from concourse.bass import ts
from concourse.bass_utils import run_bass_kernel_spmd

BF16 = ml_dtypes.bfloat16
FP8 = ml_dtypes.float8_e4m3

B, F, H, FF = 32768, 257, 400, 600
NCORES = 8
BPC = B // NCORES          # 4096 batch rows per core
Hp, FFp, Fp = 512, 640, 384  # padded feature dims
XH1 = 769                  # [x(257) | h1(400) | pad(112)] rows; 6 zr chunks + aligned h1 view at 257
ZR2K = 896                 # [g1(400) | h2(400) | pad(96)] -> 7 chunks
ZRM = 800                  # contiguous [z(400) | r(400)] output cols -> 7 M chunks
ZRC = 7
NB = 512                   # matmul free-dim tile (one PSUM bank of fp32)

AF = mybir.ActivationFunctionType
ALU = mybir.AluOpType

# packed bias column layout: name -> (offset, n_chunks)
BIAS_LAYOUT = {}
_off = 0
for _n, _c in (("bzr1", 7), ("bnx1", 4), ("bnh1", 4),
               ("bzr2", 7), ("bnx2", 4), ("bnh2", 4),
               ("bfc2", 5), ("bfc3", 5), ("bfc4", 3)):
    BIAS_LAYOUT[_n] = (_off, _c)
    _off += _c
BIAS_COLS = _off


def _pad2(a, rows, cols):
    out = np.zeros((rows, cols), dtype=np.float64)
    out[: a.shape[0], : a.shape[1]] = a
    return out


def _bias_tile(vec, padded):
    """Pack a [padded] bias vector as [128, padded//128] fp32 (partition-major)."""
    v = np.zeros(padded, dtype=np.float64)
    v[: vec.shape[0]] = vec
    return np.ascontiguousarray(v.reshape(padded // 128, 128).T).astype(np.float32)


def prepare_weights(inp):
    f64 = {k: np.asarray(v, dtype=np.float64) for k, v in inp.items()}
    w = {}

    # fc1 fold for GRU1 input side
    Wx = {}
    bx = {}
    for name in ("z", "r", "n"):
        Wx[name] = (f64[f"Wi{name}1"] @ f64["Wfc1"]).T          # [F, H]
        bx[name] = f64[f"bi{name}1"] + f64[f"Wi{name}1"] @ f64["bfc1"]

    # GRU1 z,r: K-concat [x(257) | h1(400)] -> rows 0..656 of XH1 space,
    # M = contiguous [z(400) | r(400)] = 800 -> 7 chunks; r is lane-realigned
    # on chip by a small SBUF->SBUF DMA after the sigmoid.
    Wzr1 = np.zeros((768, ZRM), dtype=np.float64)
    for g, name in enumerate(("z", "r")):
        Wzr1[:F, g * H : g * H + H] = Wx[name]
        Wzr1[F : F + H, g * H : g * H + H] = f64[f"Wh{name}1"].T
    w["Wzr1"] = Wzr1
    # GRU1 n input side: K = x chunks of XH1 (rows 0..383; rows 257+ are h1 -> zero)
    w["Wn1x"] = _pad2(Wx["n"], Fp, Hp)
    # GRU1 n hidden side: aligned h1 (XH1 rows 257..768)
    w["Wn1h"] = _pad2(f64["Whn1"].T, Hp, Hp)

    # GRU2 z,r: K-concat [g1(400) | h2(400)] -> 800 rows -> 7 chunks
    Wzr2 = np.zeros((ZR2K, ZRM), dtype=np.float64)
    for g, name in enumerate(("z", "r")):
        Wzr2[:H, g * H : g * H + H] = f64[f"Wi{name}2"].T
        Wzr2[H : 2 * H, g * H : g * H + H] = f64[f"Wh{name}2"].T
    w["Wzr2"] = Wzr2
    # GRU2 n input side: K = g1 aligned (4 chunks; chunk 3 partitions 16.. are h2 -> zero)
    w["Wn2x"] = _pad2(f64["Win2"].T, Hp, Hp)
    w["Wn2h"] = _pad2(f64["Whn2"].T, Hp, Hp)

    w["Wfc2T"] = _pad2(f64["Wfc2"].T, Hp, FFp)    # [512, 640]
    w["Wfc3T"] = _pad2(f64["Wfc3"].T, FFp, FFp)   # [640, 640]
    w["Wfc4T"] = _pad2(f64["Wfc4"].T, FFp, Fp)    # [640, 384]

    fp8_names = {"Wzr1", "Wn1x", "Wn1h", "Wzr2", "Wn2x", "Wn2h"}
    weights = {
        k: np.ascontiguousarray(v).astype(FP8 if k in fp8_names else BF16)
        for k, v in w.items()
    }

    parts = [
        ("bzr1", _bias_tile(np.concatenate([bx["z"] + f64["bhz1"],
                                            bx["r"] + f64["bhr1"]]), 896)),
        ("bnx1", _bias_tile(bx["n"], Hp)),
        ("bnh1", _bias_tile(f64["bhn1"], Hp)),
        ("bzr2", _bias_tile(np.concatenate([f64["biz2"] + f64["bhz2"],
                                            f64["bir2"] + f64["bhr2"]]), 896)),
        ("bnx2", _bias_tile(f64["bin2"], Hp)),
        ("bnh2", _bias_tile(f64["bhn2"], Hp)),
        ("bfc2", _bias_tile(f64["bfc2"], FFp)),
        ("bfc3", _bias_tile(f64["bfc3"], FFp)),
        ("bfc4", _bias_tile(f64["bfc4"], Fp)),
    ]
    biases = {"biasT": np.concatenate([p[1] for p in parts], axis=1)}
    return weights, biases


def build_nc(nbt=BPC, nb=NB):
    """Build the per-core Bass program. nbt = per-core batch, nb = free-dim tile."""
    nc = bacc.Bacc("TRN2", target_bir_lowering=False, debug=False)
    bf = mybir.dt.bfloat16
    f32 = mybir.dt.float32

    f8 = mybir.dt.float8e4

    # xh8 rows: 0..256 = x.T, 257..656 = h1.T, 657..768 = zeros (fp8 matmul
    # operand). zr view = rows 0..767 (6 chunks); aligned-h1 view = 257..768.
    xh8 = nc.declare_dram_parameter("xh8", [XH1, nbt], f8, isOutput=False)
    h1T = nc.declare_dram_parameter("h1T", [Hp, nbt], bf, isOutput=False)
    h2T = nc.declare_dram_parameter("h2T", [Hp, nbt], bf, isOutput=False)
    # h28: fp8 h2 for matmuls; aligned view + shifted views for [g1|h2] chunks.
    h28 = nc.declare_dram_parameter("h28", [Hp, nbt], f8, isOutput=False)
    wd = {}
    for name, k, m, dt_ in (
        ("Wzr1", 768, ZRM, f8), ("Wn1x", Fp, Hp, f8), ("Wn1h", Hp, Hp, f8),
        ("Wzr2", ZR2K, ZRM, f8), ("Wn2x", Hp, Hp, f8), ("Wn2h", Hp, Hp, f8),
        ("Wfc2T", Hp, FFp, bf), ("Wfc3T", FFp, FFp, bf), ("Wfc4T", FFp, Fp, bf),
    ):
        wd[name] = nc.declare_dram_parameter(name, [k, m], dt_, isOutput=False)
    biasT_d = nc.declare_dram_parameter("biasT", [128, BIAS_COLS], f32, isOutput=False)
    outT = nc.declare_dram_parameter("outT", [Fp, nbt], bf, isOutput=True)

    n_tiles = nbt // nb
    HC = Hp // 128  # 4 M-chunks per gate

    with tile.TileContext(nc) as tc:
        with (
            tc.tile_pool(name="wpool", bufs=1) as wpool,
            tc.tile_pool(name="bpool", bufs=1) as bpool,
            tc.tile_pool(name="io", bufs=2) as io,
            tc.tile_pool(name="inp", bufs=3) as inp,
            tc.tile_pool(name="act", bufs=3) as act,
            tc.tile_pool(name="psum", bufs=2, space="PSUM") as psum,
        ):
            # ACT-table warmup: first ScalarE transcendental carries the
            # ACT_TABLE_LOAD pseudo-inst; keep it off the critical chain.
            warm = bpool.tile([128, 1], f32, tag="warm")
            nc.vector.memset(warm, 0.0)
            nc.scalar.activation(warm, warm, AF.Sigmoid)

            W, BT = {}, {}

            def load_w(name, eng=None):
                dram = wd[name]
                k, m = dram.shape
                t = wpool.tile([128, k // 128, m], dram.dtype, tag=name)
                r = dram.rearrange("(c p) m -> p c m", p=128)
                for c in range(k // 128):
                    (eng or nc.sync).dma_start(out=t[:, c, :], in_=r[:, c, :])
                W[name] = t

            def load_bias():
                biasT = bpool.tile([128, BIAS_COLS], f32, tag="biasT")
                nc.sync.dma_start(out=biasT, in_=biasT_d[:, :])
                for _n, (_o, _c) in BIAS_LAYOUT.items():
                    BT[_n] = biasT[:, _o : _o + _c]

            xh_zr = xh8[0:768, :].rearrange("(c p) n -> p c n", p=128)
            h1m_al = xh8[257 : 257 + Hp, :].rearrange("(c p) n -> p c n", p=128)
            h1_bl = h1T.rearrange("(c p) n -> p c n", p=128)
            h2_bl = h2T.rearrange("(c p) n -> p c n", p=128)
            h28_al = h28.rearrange("(c p) n -> p c n", p=128)
            h2_s0 = h28[0:112, :]                     # -> partitions 16..127 of zr2 chunk 3
            h2_s1 = h28[112:496, :].rearrange("(c p) n -> p c n", p=128)
            outT_r = outT.rearrange("(c p) n -> p c n", p=128)

            def load_inputs(t):
                sl = ts(t, nb)
                xh = inp.tile([128, 6, nb], f8, tag="xh")      # zr1/nx1 K operand
                nc.sync.dma_start(out=xh, in_=xh_zr[:, :, sl])
                h1m = inp.tile([128, HC, nb], f8, tag="h1m")   # nh1 rhs (aligned h1)
                nc.sync.dma_start(out=h1m, in_=h1m_al[:, :, sl])
                h1s = inp.tile([128, HC, nb], bf, tag="h1s")   # blend h1
                nc.sync.dma_start(out=h1s, in_=h1_bl[:, :, sl])
                h2s = inp.tile([128, HC, nb], bf, tag="h2s")   # blend h2
                nc.sync.dma_start(out=h2s, in_=h2_bl[:, :, sl])
                h28s = inp.tile([128, HC, nb], f8, tag="h28s") # nh2 rhs (aligned h2)
                nc.sync.dma_start(out=h28s, in_=h28_al[:, :, sl])
                return xh, h1m, h1s, h2s, h28s

            tile0_inputs = load_inputs(0)

            # GRU1 weights share the sync ring with the input tiles; everything
            # needed later streams on the otherwise-idle PE ring in parallel.
            load_w("Wzr1")
            load_bias()
            for name in ("Wn1x", "Wn1h"):
                load_w(name)
            for name in ("Wzr2", "Wn2x", "Wn2h", "Wfc2T", "Wfc3T", "Wfc4T"):
                load_w(name, eng=nc.scalar)

            def matseq(ps, pairs):
                n = len(pairs)
                for i, (lhsT, rhs) in enumerate(pairs):
                    nc.tensor.matmul(ps, lhsT, rhs, start=(i == 0), stop=(i == n - 1))

            DR = mybir.MatmulPerfMode.DoubleRow

            def matseq_dr(ps, Wt, kc, col, mw, rhs_t):
                """fp8 accumulation over kc K-chunks of [128, kc, *] tiles using
                DoubleRow on consecutive chunk pairs (odd tail chunk = normal)."""
                n = (kc + 1) // 2
                for i in range(n):
                    k = 2 * i
                    if k + 2 <= kc:
                        nc.tensor.matmul(
                            ps, Wt[:, k : k + 2, col : col + mw],
                            rhs_t[:, k : k + 2, :],
                            start=(i == 0), stop=(i == n - 1), perf_mode=DR)
                    else:
                        nc.tensor.matmul(
                            ps, Wt[:, k, col : col + mw], rhs_t[:, k, :],
                            start=(i == 0), stop=(i == n - 1))

            def gru(zr_t, kzr, Wzr, nx_t, knx, Wnx, nh_t, Wnh, h_al,
                    bzr, bnx, bnh, out_chunk):
                """One GRU step, all matmuls fp8/DoubleRow. zr_t/nx_t/nh_t are
                [128, kc, nb] fp8 rhs tiles; h_al: bf16 blend chunks.
                out_chunk(m) -> output AP for chunk m."""
                # z,r preactivations: 7 contiguous M chunks (chunk 6 is 32 wide)
                zro = act.tile([128, ZRC, nb], bf, tag="zro")
                for c in range(ZRC):
                    mw = min(128, ZRM - c * 128)
                    ps = psum.tile([128, nb], f32, tag="ps_zr")
                    matseq_dr(ps[:mw, :], Wzr, kzr, c * 128, mw, zr_t)
                    nc.scalar.activation(zro[:mw, c, :], ps[:mw, :], AF.Sigmoid,
                                         bias=bzr[:mw, c : c + 1])
                # realign r (features at concat rows 400+f) to h's lanes
                r_al = act.tile([128, HC, nb], bf, tag="r_al")
                for m in range(3):
                    nc.scalar.dma_start(out=r_al[0:112, m, :], in_=zro[16:128, 3 + m, :])
                    nc.scalar.dma_start(out=r_al[112:128, m, :], in_=zro[0:16, 4 + m, :])
                nc.scalar.dma_start(out=r_al[0:16, 3, :], in_=zro[16:32, 6, :])

                for m in range(HC):
                    pz = 128 if m < 3 else 16   # valid rows of this chunk
                    col = m * 128
                    psx = psum.tile([128, nb], f32, tag="ps_nx")
                    matseq_dr(psx, Wnx, knx, col, 128, nx_t)
                    psh = psum.tile([128, nb], f32, tag="ps_nh")
                    matseq_dr(psh, Wnh, HC, col, 128, nh_t)
                    # rhn = (psh + bnh) * r ; npre = (psx + bnx) + rhn ; n = tanh(npre)
                    rhn = act.tile([128, nb], f32, tag="rhn")
                    nc.vector.scalar_tensor_tensor(
                        rhn[:pz, :], psh[:pz, :], bnh[:pz, m : m + 1],
                        r_al[:pz, m, :], op0=ALU.add, op1=ALU.mult)
                    npre = act.tile([128, nb], f32, tag="npre")
                    nc.vector.scalar_tensor_tensor(
                        npre[:pz, :], psx[:pz, :], bnx[:pz, m : m + 1],
                        rhn[:pz, :], op0=ALU.add, op1=ALU.add)
                    n_t = act.tile([128, nb], bf, tag="n_t")
                    nc.scalar.activation(n_t[:pz, :], npre[:pz, :], AF.Tanh)
                    # h' = n + z*(h - n);  z chunk m lives in zro (contig layout)
                    z_ap = zro[:pz, m, :] if m < 3 else zro[0:16, 3, :]
                    d = act.tile([128, nb], bf, tag="d")
                    nc.vector.tensor_sub(d[:pz, :], h_al[m][:pz, :], n_t[:pz, :])
                    zd = act.tile([128, nb], bf, tag="zd")
                    nc.vector.tensor_mul(zd[:pz, :], z_ap, d[:pz, :])
                    out_ap = out_chunk(m)
                    p = min(out_ap.shape[0], pz)
                    nc.vector.tensor_add(out_ap[:p, :] if out_ap.shape[0] > p else out_ap,
                                         n_t[:p, :], zd[:p, :])

            def fc(in_ks, Wt, bias, mc, kind, out_tag):
                outs = io.tile([128, mc, nb], bf, tag=out_tag)
                for m in range(mc):
                    ps = psum.tile([128, nb], f32, tag="ps_fc")
                    matseq(ps, [(Wt[:, k, m * 128 : (m + 1) * 128], rhs)
                                for k, rhs in enumerate(in_ks)])
                    if kind == "relu":
                        nc.vector.tensor_scalar(
                            outs[:, m, :], ps, bias[:, m : m + 1], 0.0,
                            op0=ALU.add, op1=ALU.max)
                    else:
                        nc.scalar.activation(outs[:, m, :], ps, AF.Sigmoid,
                                             bias=bias[:, m : m + 1])
                return outs

            for t in range(n_tiles):
                sl = ts(t, nb)
                xh, h1m, h1s, h2s, h28s = \
                    tile0_inputs if t == 0 else load_inputs(t)

                # zr2op = GRU2's [g1|h2] fp8 operand (one tile so DoubleRow can
                # pair consecutive chunks): chunks 0..2 + [0:16] of chunk 3 are
                # written by GRU1's blend (fp8 out); the rest comes from h28.
                zr2op = io.tile([128, ZRC, nb], f8, tag="zr2op")
                nc.sync.dma_start(out=zr2op[16:128, 3, :], in_=h2_s0[:, sl])
                nc.sync.dma_start(out=zr2op[:, 4:7, :], in_=h2_s1[:, :, sl])

                def g1_out(m):
                    return zr2op[:, m, :] if m < 3 else zr2op[0:16, 3, :]

                h1_ks = [h1s[:, c, :] for c in range(HC)]
                gru(xh, 6, W["Wzr1"], xh, 3, W["Wn1x"], h1m, W["Wn1h"], h1_ks,
                    BT["bzr1"], BT["bnx1"], BT["bnh1"], g1_out)

                h2_ks = [h2s[:, c, :] for c in range(HC)]
                g2 = io.tile([128, HC, nb], bf, tag="g2")
                # g2 pad rows (feature >= 400 of chunk 3) must be finite for
                # fc2's zero-weight contraction: zero them once per tile.
                nc.gpsimd.memset(g2[:, 3, :], 0.0)
                gru(zr2op, ZRC, W["Wzr2"], zr2op, HC, W["Wn2x"], h28s, W["Wn2h"], h2_ks,
                    BT["bzr2"], BT["bnx2"], BT["bnh2"],
                    lambda m: g2[:, m, :])

                g2_ks = [g2[:, c, :] for c in range(HC)]
                f2 = fc(g2_ks, W["Wfc2T"], BT["bfc2"], FFp // 128, "relu", "f2")
                f3 = fc([f2[:, c, :] for c in range(FFp // 128)],
                        W["Wfc3T"], BT["bfc3"], FFp // 128, "relu", "f3")
                o = fc([f3[:, c, :] for c in range(FFp // 128)],
                       W["Wfc4T"], BT["bfc4"], Fp // 128, "sig", "o")
                nc.sync.dma_start(out=outT_r[:, :, sl], in_=o)

    nc.compile()
    return nc


def _shard_inputs(inp, weights, biases):
    x = np.asarray(inp["x"], dtype=np.float32)
    h1 = np.asarray(inp["h1"], dtype=np.float32)
    h2 = np.asarray(inp["h2"], dtype=np.float32)

    xh8 = np.zeros((NCORES, XH1, BPC), dtype=FP8)    # matmul operand [x|h1]
    h1T = np.zeros((NCORES, Hp, BPC), dtype=BF16)    # blend h1
    h2T = np.zeros((NCORES, Hp, BPC), dtype=BF16)    # blend h2
    h28 = np.zeros((NCORES, Hp, BPC), dtype=FP8)     # matmul h2
    for i in range(NCORES):
        sl = slice(i * BPC, (i + 1) * BPC)
        xh8[i, :F] = x[sl].T.astype(FP8)
        xh8[i, F : F + H] = h1[sl].T.astype(FP8)
        h1T[i, :H] = h1[sl].T.astype(BF16)
        h2T[i, :H] = h2[sl].T.astype(BF16)
        h28[i, :H] = h2[sl].T.astype(FP8)

    in_maps = []
    for i in range(NCORES):
        m = {"xh8": xh8[i], "h1T": h1T[i], "h2T": h2T[i], "h28": h28[i]}
        m.update(weights)
        m.update(biases)
        in_maps.append(m)
    return in_maps


def _run(inp, trace=False):
    weights, biases = prepare_weights(inp)
    nc = build_nc()
    in_maps = _shard_inputs(inp, weights, biases)
    res = run_bass_kernel_spmd(nc, in_maps, list(range(NCORES)), trace=trace)
    out = np.empty((B, F), dtype=np.float32)
    for i in range(NCORES):
        out[i * BPC : (i + 1) * BPC] = (
            np.asarray(res.results[i]["outT"][:F]).astype(np.float32).T
        )
    return out, res


def kernel(**inputs) -> np.ndarray:
    out, _ = _run(inputs, trace=False)
    return out



# revision 5
# speedup vs baseline: 1.1765x; 1.1765x over previous
"""NsNet2 single-step (fc1 + 2x GRU cell + 3x FC) Trainium2 kernel.

Strategy:
  - Pure data parallel: batch B=32768 sharded as 4096 rows per NeuronCore (8 cores).
  - Feature-major layout on chip: activations live as [feat, batch]; host
    transposes inputs/outputs (free; off the HW critical path).
  - ALL matmuls fp8(e4m3)+DoubleRow, fp32 PSUM. Weights are scaled by S=16 on
    the host to lift them out of the fp8 denormal range; the scale is divided
    back out for free via ScalarE activation `scale` or the stt bias slot.
  - fc1 folded into GRU1 input-gate weights (fc1 is linear, f1 only feeds GRU1).
  - z,r gates K-concat their input and hidden operands ([x|h1] resp. [g1|h2])
    and M-concat z|r into one 800-col group (7 chunks).
  - The n-gate hidden matmuls reuse the SAME SBUF operand as z,r via shifted
    chunk views (weights re-laid to match), so only one fp8 operand stream is
    loaded per GRU.
  - 5-stage software pipeline over batch tiles (GRU1 | GRU2 | fc2 | fc3 | fc4)
    so the FIFO Tensor queue never head-of-line blocks on the elementwise
    chain of the same tile.
  - Elementwise work spread over ScalarE (sigmoid/tanh/fc3-relu/fc4-sigmoid),
    VectorE (stt chains, fc2-relu, blend mul/add) and GpSimd (blend sub).
"""

import os
import sys

import numpy as np
import ml_dtypes

sys.path.insert(0, "/opt/trn_rl_repo")

import concourse.bacc as bacc
import concourse.bass as bass
import concourse.mybir as mybir
import concourse.tile as tile
from concourse.bass import ts
from concourse.bass_utils import run_bass_kernel_spmd

BF16 = ml_dtypes.bfloat16
FP8 = ml_dtypes.float8_e4m3

B, F, H, FF = 32768, 257, 400, 600
NCORES = 8
BPC = B // NCORES          # 4096 batch rows per core
Hp, FFp, Fp = 512, 640, 384  # padded feature dims
XHK = 768                  # [x(257) | h1(400) | pad(111)] -> 6 zr K chunks
ZR2K = 896                 # [g1(400) | h2(400) | pad(96)] -> 7 chunks
ZRM = 800                  # contiguous [z(400) | r(400)] output cols -> 7 M chunks
ZRC = 7
NB = 512                   # matmul free-dim tile (one PSUM bank of fp32)
S = 16.0                   # fp8 weight scale (denormal avoidance)

AF = mybir.ActivationFunctionType
ALU = mybir.AluOpType

# packed bias column layout: name -> (offset, n_chunks)
BIAS_LAYOUT = {}
_off = 0
for _n, _c in (("bzr1", 7), ("bnx1", 4), ("bnh1", 4),
               ("bzr2", 7), ("bnx2", 4), ("bnh2", 4),
               ("bfc2", 5), ("bfc3", 5), ("bfc4", 3)):
    BIAS_LAYOUT[_n] = (_off, _c)
    _off += _c
BIAS_COLS = _off


def _pad2(a, rows, cols, r0=0):
    out = np.zeros((rows, cols), dtype=np.float64)
    out[r0 : r0 + a.shape[0], : a.shape[1]] = a
    return out


def _bias_tile(vec, padded):
    """Pack a [padded] bias vector as [128, padded//128] fp32 (partition-major)."""
    v = np.zeros(padded, dtype=np.float64)
    v[: vec.shape[0]] = vec
    return np.ascontiguousarray(v.reshape(padded // 128, 128).T).astype(np.float32)


def prepare_weights(inp):
    f64 = {k: np.asarray(v, dtype=np.float64) for k, v in inp.items()}
    w = {}

    # fc1 fold for GRU1 input side
    Wx = {}
    bx = {}
    for name in ("z", "r", "n"):
        Wx[name] = (f64[f"Wi{name}1"] @ f64["Wfc1"]).T          # [F, H]
        bx[name] = f64[f"bi{name}1"] + f64[f"Wi{name}1"] @ f64["bfc1"]

    # GRU1 z,r: K-concat [x(257) | h1(400)] rows 0..656 of 768, M = [z|r] = 800
    Wzr1 = np.zeros((XHK, ZRM), dtype=np.float64)
    for g, name in enumerate(("z", "r")):
        Wzr1[:F, g * H : g * H + H] = Wx[name]
        Wzr1[F : F + H, g * H : g * H + H] = f64[f"Wh{name}1"].T
    w["Wzr1"] = Wzr1
    # GRU1 n input side: K = xh chunks 0..2 (rows 0..383; rows 257+ are h1 -> 0)
    w["Wn1x"] = _pad2(Wx["n"], Fp, Hp)
    # GRU1 n hidden side: K = xh chunks 2..5 (rows 256..767); h1 lives at 257..656
    w["Wn1h"] = _pad2(f64["Whn1"].T, Hp, Hp, r0=1)

    # GRU2 z,r: K-concat [g1(400) | h2(400)] -> 800 rows -> 7 chunks
    Wzr2 = np.zeros((ZR2K, ZRM), dtype=np.float64)
    for g, name in enumerate(("z", "r")):
        Wzr2[:H, g * H : g * H + H] = f64[f"Wi{name}2"].T
        Wzr2[H : 2 * H, g * H : g * H + H] = f64[f"Wh{name}2"].T
    w["Wzr2"] = Wzr2
    # GRU2 n input side: K = zr2op chunks 0..3 (rows 0..511; g1 at 0..399)
    w["Wn2x"] = _pad2(f64["Win2"].T, Hp, Hp)
    # GRU2 n hidden side: K = zr2op chunks 3..6 (rows 384..895); h2 at 400..799
    w["Wn2h"] = _pad2(f64["Whn2"].T, Hp, Hp, r0=16)

    w["Wfc2T"] = _pad2(f64["Wfc2"].T, Hp, FFp)    # [512, 640]
    w["Wfc3T"] = _pad2(f64["Wfc3"].T, FFp, FFp)   # [640, 640]
    w["Wfc4T"] = _pad2(f64["Wfc4"].T, FFp, Fp)    # [640, 384]

    weights = {
        k: np.ascontiguousarray(S * v).astype(FP8) for k, v in w.items()
    }

    parts = [
        ("bzr1", _bias_tile(np.concatenate([bx["z"] + f64["bhz1"],
                                            bx["r"] + f64["bhr1"]]), 896)),
        ("bnx1", _bias_tile(S * bx["n"], Hp)),
        ("bnh1", _bias_tile(S * f64["bhn1"], Hp)),
        ("bzr2", _bias_tile(np.concatenate([f64["biz2"] + f64["bhz2"],
                                            f64["bir2"] + f64["bhr2"]]), 896)),
        ("bnx2", _bias_tile(S * f64["bin2"], Hp)),
        ("bnh2", _bias_tile(S * f64["bhn2"], Hp)),
        ("bfc2", _bias_tile(S * f64["bfc2"], FFp)),
        ("bfc3", _bias_tile(S * f64["bfc3"], FFp)),
        ("bfc4", _bias_tile(f64["bfc4"], Fp)),
    ]
    biases = {"biasT": np.concatenate([p[1] for p in parts], axis=1)}
    return weights, biases


def build_nc(nbt=BPC, nb=NB):
    """Build the per-core Bass program. nbt = per-core batch, nb = free-dim tile."""
    nc = bacc.Bacc("TRN2", target_bir_lowering=False, debug=False)
    bf = mybir.dt.bfloat16
    f32 = mybir.dt.float32
    f8 = mybir.dt.float8e4

    xh8 = nc.declare_dram_parameter("xh8", [XHK, nbt], f8, isOutput=False)
    h1T = nc.declare_dram_parameter("h1T", [Hp, nbt], bf, isOutput=False)
    h2T = nc.declare_dram_parameter("h2T", [Hp, nbt], bf, isOutput=False)
    h28 = nc.declare_dram_parameter("h28", [Hp, nbt], f8, isOutput=False)
    wd = {}
    for name, k, m in (
        ("Wzr1", XHK, ZRM), ("Wn1x", Fp, Hp), ("Wn1h", Hp, Hp),
        ("Wzr2", ZR2K, ZRM), ("Wn2x", Hp, Hp), ("Wn2h", Hp, Hp),
        ("Wfc2T", Hp, FFp), ("Wfc3T", FFp, FFp), ("Wfc4T", FFp, Fp),
    ):
        wd[name] = nc.declare_dram_parameter(name, [k, m], f8, isOutput=False)
    biasT_d = nc.declare_dram_parameter("biasT", [128, BIAS_COLS], f32, isOutput=False)
    outT = nc.declare_dram_parameter("outT", [Fp, nbt], bf, isOutput=True)

    n_tiles = nbt // nb
    HC = Hp // 128  # 4 M-chunks per gate
    DR = mybir.MatmulPerfMode.DoubleRow

    with tile.TileContext(nc) as tc:
        with (
            tc.tile_pool(name="wpool", bufs=1) as wpool,
            tc.tile_pool(name="bpool", bufs=1) as bpool,
            tc.tile_pool(name="inp2", bufs=2) as inp2,
            tc.tile_pool(name="inp3", bufs=3) as inp3,
            tc.tile_pool(name="io", bufs=3) as io,
            tc.tile_pool(name="act2", bufs=2) as act2,
            tc.tile_pool(name="act3", bufs=3) as act3,
            tc.tile_pool(name="psum", bufs=2, space="PSUM") as psum,
        ):
            # ACT-table warmup: first ScalarE transcendental carries the
            # ACT_TABLE_LOAD pseudo-inst; keep it off the critical chain.
            warm = bpool.tile([128, 1], f32, tag="warm")
            nc.vector.memset(warm, 0.0)
            nc.scalar.activation(warm, warm, AF.Sigmoid)

            W, BT = {}, {}

            def load_w(name, eng):
                dram = wd[name]
                k, m = dram.shape
                t = wpool.tile([128, k // 128, m], dram.dtype, tag=name)
                r = dram.rearrange("(c p) m -> p c m", p=128)
                for c in range(k // 128):
                    eng.dma_start(out=t[:, c, :], in_=r[:, c, :])
                W[name] = t

            def load_bias():
                biasT = bpool.tile([128, BIAS_COLS], f32, tag="biasT")
                nc.sync.dma_start(out=biasT, in_=biasT_d[:, :])
                for _n, (_o, _c) in BIAS_LAYOUT.items():
                    BT[_n] = biasT[:, _o : _o + _c]

            xh_r = xh8.rearrange("(c p) n -> p c n", p=128)
            h1_bl = h1T.rearrange("(c p) n -> p c n", p=128)
            h2_bl = h2T.rearrange("(c p) n -> p c n", p=128)
            h2_s0 = h28[0:112, :]                     # -> partitions 16..127 of zr2 chunk 3
            h2_s1 = h28[112:496, :].rearrange("(c p) n -> p c n", p=128)
            outT_r = outT.rearrange("(c p) n -> p c n", p=128)

            ST = [dict() for _ in range(n_tiles)]

            def load_inputs(t):
                sl = ts(t, nb)
                xh = inp2.tile([128, 6, nb], f8, tag="xh")      # zr1/nx1/nh1 K operand
                nc.sync.dma_start(out=xh, in_=xh_r[:, :, sl])
                h1s = inp2.tile([128, HC, nb], bf, tag="h1s")   # blend h1
                nc.sync.dma_start(out=h1s, in_=h1_bl[:, :, sl])
                h2s = inp3.tile([128, HC, nb], bf, tag="h2s")   # blend h2
                nc.sync.dma_start(out=h2s, in_=h2_bl[:, :, sl])
                ST[t]["xh"], ST[t]["h1s"], ST[t]["h2s"] = xh, h1s, h2s

            def matseq_dr(ps, Wt, kc, col, mw, rhs_t, r0=0):
                """fp8 accumulation over kc K-chunks using DoubleRow on
                consecutive chunk pairs (odd tail chunk = normal). rhs chunks
                start at r0 within the operand tile."""
                n = (kc + 1) // 2
                for i in range(n):
                    k = 2 * i
                    if k + 2 <= kc:
                        nc.tensor.matmul(
                            ps, Wt[:, k : k + 2, col : col + mw],
                            rhs_t[:, r0 + k : r0 + k + 2, :],
                            start=(i == 0), stop=(i == n - 1), perf_mode=DR)
                    else:
                        nc.tensor.matmul(
                            ps, Wt[:, k, col : col + mw], rhs_t[:, r0 + k, :],
                            start=(i == 0), stop=(i == n - 1))

            def gru(rhs, Wzr, zr_kc, Wnx, nx_kc, Wnh, nh_r0, bzr, bnx, bnh,
                    h_bl, out_full, out_fat3=None, out_sliver=None):
                """One GRU step. rhs: single fp8 K-operand tile; n-gate x side
                reads chunks 0.., hidden side chunks nh_r0.. of rhs. Output h':
                either out_full [128,4,nb] (pad rows compute exact zeros), or
                out_fat3 (chunks 0..2) + out_sliver (chunk 3 rows 0..15) when
                the target's chunk-3 pad lanes must not be written.

                All chain ops run full-lane: pad lanes of every operand are
                exact zeros (zero weights -> zero psum; r_al pad memset), so
                pads propagate as zeros at no extra instruction cost."""
                # z,r preactivations: 7 contiguous M chunks (chunk 6 is 32 wide)
                zro = act3.tile([128, ZRC, nb], bf, tag="zro")
                for c in range(ZRC):
                    mw = min(128, ZRM - c * 128)
                    ps = psum.tile([128, nb], f32, tag="ps_zr")
                    matseq_dr(ps[:mw, :], Wzr, zr_kc, c * 128, mw, rhs)
                    nc.scalar.activation(zro[:mw, c, :], ps[:mw, :], AF.Sigmoid,
                                         bias=bzr[:mw, c : c + 1], scale=1.0 / S)
                # realign r (features at concat rows 400+f) to h's lanes
                r_al = act3.tile([128, HC, nb], bf, tag="r_al")
                nc.gpsimd.memset(r_al[:, 3, :], 0.0)
                nc.sync.dma_start(out=r_al[0:112, 0:3, :], in_=zro[16:128, 3:6, :])
                nc.sync.dma_start(out=r_al[112:128, 0:3, :], in_=zro[0:16, 4:7, :])
                nc.sync.dma_start(out=r_al[0:16, 3, :], in_=zro[16:32, 6, :])

                npre = act2.tile([128, HC, nb], f32, tag="npre")
                for m in range(HC):
                    col = m * 128
                    psx = psum.tile([128, nb], f32, tag="ps_nx")
                    matseq_dr(psx, Wnx, nx_kc, col, 128, rhs)
                    psh = psum.tile([128, nb], f32, tag="ps_nh")
                    matseq_dr(psh, Wnh, HC, col, 128, rhs, r0=nh_r0)
                    # rhn = (psh + bnh) * r ; npre = (psx + bnx) + rhn
                    rhn = act3.tile([128, nb], f32, tag="rhn")
                    nc.vector.scalar_tensor_tensor(
                        rhn, psh, bnh[:, m : m + 1],
                        r_al[:, m, :], op0=ALU.add, op1=ALU.mult)
                    nc.vector.scalar_tensor_tensor(
                        npre[:, m, :], psx, bnx[:, m : m + 1],
                        rhn, op0=ALU.add, op1=ALU.add)
                # n = tanh(npre/S), one fat ACT over all 4 chunks
                n_t = act3.tile([128, HC, nb], bf, tag="n_t")
                nc.scalar.activation(n_t, npre, AF.Tanh, scale=1.0 / S)
                # h' = n + z*(h - n); z of chunk 3 rows 16+ is r-junk but meets
                # d==0 there, so pads still come out zero.
                d = act2.tile([128, HC, nb], bf, tag="d")
                nc.gpsimd.tensor_sub(d, h_bl, n_t)
                zd = act2.tile([128, HC, nb], bf, tag="zd")
                nc.vector.tensor_mul(zd, zro[:, 0:HC, :], d)
                if out_full is not None:
                    nc.vector.tensor_add(out_full, n_t, zd)
                else:
                    nc.vector.tensor_add(out_fat3, n_t[:, 0:3, :], zd[:, 0:3, :])
                    nc.vector.tensor_add(out_sliver, n_t[0:16, 3, :], zd[0:16, 3, :])

            def stage_gru1(t):
                sl = ts(t, nb)
                # zr2op = GRU2's [g1|h2] fp8 operand: g1 written by GRU1 blend,
                # h2 stitched in by DMA at concat rows 400..799.
                zr2op = io.tile([128, ZRC, nb], f8, tag="zr2op")
                nc.sync.dma_start(out=zr2op[16:128, 3, :], in_=h2_s0[:, sl])
                nc.sync.dma_start(out=zr2op[:, 4:7, :], in_=h2_s1[:, :, sl])
                ST[t]["zr2op"] = zr2op
                gru(ST[t]["xh"], W["Wzr1"], 6, W["Wn1x"], 3, W["Wn1h"], 2,
                    BT["bzr1"], BT["bnx1"], BT["bnh1"], ST[t]["h1s"],
                    None, zr2op[:, 0:3, :], zr2op[0:16, 3, :])

            def stage_gru2(t):
                zr2op = ST[t]["zr2op"]
                g2 = io.tile([128, HC, nb], f8, tag="g2")
                gru(zr2op, W["Wzr2"], ZRC, W["Wn2x"], HC, W["Wn2h"], 3,
                    BT["bzr2"], BT["bnx2"], BT["bnh2"], ST[t]["h2s"], g2)
                ST[t]["g2"] = g2

            def stage_fc2(t):
                g2 = ST[t].pop("g2")
                f2 = io.tile([128, FFp // 128, nb], f8, tag="f2")
                for m in range(FFp // 128):
                    ps = psum.tile([128, nb], f32, tag="ps_fc")
                    matseq_dr(ps, W["Wfc2T"], HC, m * 128, 128, g2)
                    # f2 = relu(ps + S*b) = S*relu(W g2 + b)
                    nc.vector.tensor_scalar(
                        f2[:, m, :], ps, BT["bfc2"][:, m : m + 1], 0.0,
                        op0=ALU.add, op1=ALU.max)
                ST[t]["f2"] = f2

            def stage_fc3(t):
                f2 = ST[t].pop("f2")
                f3 = io.tile([128, FFp // 128, nb], f8, tag="f3")
                for m in range(FFp // 128):
                    ps = psum.tile([128, nb], f32, tag="ps_fc")
                    matseq_dr(ps, W["Wfc3T"], FFp // 128, m * 128, 128, f2)
                    # f3 = relu(ps/S + S*b) = S*relu(W f2 + b)
                    nc.scalar.activation(f3[:, m, :], ps, AF.Relu,
                                         bias=BT["bfc3"][:, m : m + 1],
                                         scale=1.0 / S)
                ST[t]["f3"] = f3

            def stage_fc4(t):
                sl = ts(t, nb)
                f3 = ST[t].pop("f3")
                o = io.tile([128, Fp // 128, nb], bf, tag="o")
                for m in range(Fp // 128):
                    ps = psum.tile([128, nb], f32, tag="ps_fc")
                    matseq_dr(ps, W["Wfc4T"], FFp // 128, m * 128, 128, f3)
                    nc.scalar.activation(o[:, m, :], ps, AF.Sigmoid,
                                         bias=BT["bfc4"][:, m : m + 1],
                                         scale=1.0 / (S * S))
                nc.sync.dma_start(out=outT_r[:, :, sl], in_=o)

            # startup loads: sync carries GRU1 weights + first inputs; GRU2
            # weights go on the (otherwise idle at start) gpsimd ring; FC
            # weights on the scalar ring (needed only from iteration 2 on).
            load_inputs(0)
            load_w("Wzr1", nc.sync)
            load_bias()
            load_w("Wn1x", nc.sync)
            load_w("Wn1h", nc.sync)
            for name in ("Wzr2", "Wn2x", "Wn2h"):
                load_w(name, nc.gpsimd)
            for name in ("Wfc2T", "Wfc3T", "Wfc4T"):
                load_w(name, nc.scalar)

            PF = 1  # input prefetch depth (iterations ahead)
            for i in range(n_tiles + 4):
                if i < n_tiles:
                    if i + PF < n_tiles:
                        load_inputs(i + PF)
                    stage_gru1(i)
                if 0 <= i - 1 < n_tiles:
                    stage_gru2(i - 1)
                if 0 <= i - 2 < n_tiles:
                    stage_fc2(i - 2)
                if 0 <= i - 3 < n_tiles:
                    stage_fc3(i - 3)
                if 0 <= i - 4 < n_tiles:
                    stage_fc4(i - 4)

    nc.compile()
    return nc


def _shard_inputs(inp, weights, biases):
    x = np.asarray(inp["x"], dtype=np.float32)
    h1 = np.asarray(inp["h1"], dtype=np.float32)
    h2 = np.asarray(inp["h2"], dtype=np.float32)

    xh8 = np.zeros((NCORES, XHK, BPC), dtype=FP8)    # matmul operand [x|h1]
    h1T = np.zeros((NCORES, Hp, BPC), dtype=BF16)    # blend h1
    h2T = np.zeros((NCORES, Hp, BPC), dtype=BF16)    # blend h2
    h28 = np.zeros((NCORES, Hp, BPC), dtype=FP8)     # zr2op h2 stitch source
    for i in range(NCORES):
        sl = slice(i * BPC, (i + 1) * BPC)
        xh8[i, :F] = x[sl].T.astype(FP8)
        xh8[i, F : F + H] = h1[sl].T.astype(FP8)
        h1T[i, :H] = h1[sl].T.astype(BF16)
        h2T[i, :H] = h2[sl].T.astype(BF16)
        h28[i, :H] = h2[sl].T.astype(FP8)

    in_maps = []
    for i in range(NCORES):
        m = {"xh8": xh8[i], "h1T": h1T[i], "h2T": h2T[i], "h28": h28[i]}
        m.update(weights)
        m.update(biases)
        in_maps.append(m)
    return in_maps


def _run(inp, trace=False):
    weights, biases = prepare_weights(inp)
    nc = build_nc()
    in_maps = _shard_inputs(inp, weights, biases)
    res = run_bass_kernel_spmd(nc, in_maps, list(range(NCORES)), trace=trace)
    out = np.empty((B, F), dtype=np.float32)
    for i in range(NCORES):
        out[i * BPC : (i + 1) * BPC] = (
            np.asarray(res.results[i]["outT"][:F]).astype(np.float32).T
        )
    return out, res


def kernel(**inputs) -> np.ndarray:
    out, _ = _run(inputs, trace=False)
    return out


# revision 8
# speedup vs baseline: 1.1948x; 1.0155x over previous
"""NsNet2 single-step (fc1 + 2x GRU cell + 3x FC) Trainium2 kernel.

Strategy:
  - Pure data parallel: batch B=32768 sharded as 4096 rows per NeuronCore (8 cores).
  - Feature-major layout on chip: activations live as [feat, batch]; host
    transposes inputs/outputs (free; off the HW critical path).
  - ALL matmuls fp8(e4m3)+DoubleRow, fp32 PSUM. Weights are scaled by S=16 on
    the host to lift them out of the fp8 denormal range; the scale is divided
    back out for free via ScalarE activation `scale` or the stt bias slot.
  - fc1 folded into GRU1 input-gate weights (fc1 is linear, f1 only feeds GRU1).
  - z,r gates K-concat their input and hidden operands ([x|h1] resp. [g1|h2])
    and M-concat z|r into one 800-col group (7 chunks).
  - The n-gate hidden matmuls reuse the SAME SBUF operand as z,r via shifted
    chunk views (weights re-laid to match), so only one fp8 operand stream is
    loaded per GRU.
  - 5-stage software pipeline over batch tiles (GRU1 | GRU2 | fc2 | fc3 | fc4)
    so the FIFO Tensor queue never head-of-line blocks on the elementwise
    chain of the same tile.
  - Elementwise work spread over ScalarE (sigmoid/tanh/fc3-relu/fc4-sigmoid),
    VectorE (stt chains, fc2-relu, blend mul/add) and GpSimd (blend sub).
"""

import os
import sys

import numpy as np
import ml_dtypes

sys.path.insert(0, "/opt/trn_rl_repo")

import concourse.bacc as bacc
import concourse.bass as bass
import concourse.mybir as mybir
import concourse.tile as tile
from concourse.bass import ts
from concourse.bass_utils import run_bass_kernel_spmd

BF16 = ml_dtypes.bfloat16
FP8 = ml_dtypes.float8_e4m3

B, F, H, FF = 32768, 257, 400, 600
NCORES = 8
BPC = B // NCORES          # 4096 batch rows per core
Hp, FFp, Fp = 512, 640, 384  # padded feature dims
XHK = 768                  # [x(257) | h1(400) | pad(111)] -> 6 zr K chunks
ZR2K = 896                 # [g1(400) | h2(400) | pad(96)] -> 7 chunks
ZRM = 800                  # contiguous [z(400) | r(400)] output cols -> 7 M chunks
ZRC = 7
NB = 512                   # matmul free-dim tile (one PSUM bank of fp32)
S = 16.0                   # fp8 weight scale (denormal avoidance)

AF = mybir.ActivationFunctionType
ALU = mybir.AluOpType

# packed bias column layout: name -> (offset, n_chunks)
BIAS_LAYOUT = {}
_off = 0
for _n, _c in (("bzr1", 7), ("bnx1", 4), ("bnh1", 4),
               ("bzr2", 7), ("bnx2", 4), ("bnh2", 4),
               ("bfc2", 5), ("bfc3", 5), ("bfc4", 3)):
    BIAS_LAYOUT[_n] = (_off, _c)
    _off += _c
BIAS_COLS = _off


def _pad2(a, rows, cols, r0=0):
    out = np.zeros((rows, cols), dtype=np.float64)
    out[r0 : r0 + a.shape[0], : a.shape[1]] = a
    return out


def _bias_tile(vec, padded):
    """Pack a [padded] bias vector as [128, padded//128] fp32 (partition-major)."""
    v = np.zeros(padded, dtype=np.float64)
    v[: vec.shape[0]] = vec
    return np.ascontiguousarray(v.reshape(padded // 128, 128).T).astype(np.float32)


def prepare_weights(inp):
    f64 = {k: np.asarray(v, dtype=np.float64) for k, v in inp.items()}
    w = {}

    # fc1 fold for GRU1 input side
    Wx = {}
    bx = {}
    for name in ("z", "r", "n"):
        Wx[name] = (f64[f"Wi{name}1"] @ f64["Wfc1"]).T          # [F, H]
        bx[name] = f64[f"bi{name}1"] + f64[f"Wi{name}1"] @ f64["bfc1"]

    # GRU1 z,r: K-concat [x(257) | h1(400)] rows 0..656 of 768, M = [z|r] = 800
    Wzr1 = np.zeros((XHK, ZRM), dtype=np.float64)
    for g, name in enumerate(("z", "r")):
        Wzr1[:F, g * H : g * H + H] = Wx[name]
        Wzr1[F : F + H, g * H : g * H + H] = f64[f"Wh{name}1"].T
    w["Wzr1"] = Wzr1
    # GRU1 n input side: K = xh chunks 0..2 (rows 0..383; rows 257+ are h1 -> 0)
    w["Wn1x"] = _pad2(Wx["n"], Fp, Hp)
    # GRU1 n hidden side: K = xh chunks 2..5 (rows 256..767); h1 lives at 257..656
    w["Wn1h"] = _pad2(f64["Whn1"].T, Hp, Hp, r0=1)

    # GRU2 z,r: K-concat [g1(400) | h2(400)] -> 800 rows -> 7 chunks
    Wzr2 = np.zeros((ZR2K, ZRM), dtype=np.float64)
    for g, name in enumerate(("z", "r")):
        Wzr2[:H, g * H : g * H + H] = f64[f"Wi{name}2"].T
        Wzr2[H : 2 * H, g * H : g * H + H] = f64[f"Wh{name}2"].T
    w["Wzr2"] = Wzr2
    # GRU2 n input side: K = zr2op chunks 0..3 (rows 0..511; g1 at 0..399)
    w["Wn2x"] = _pad2(f64["Win2"].T, Hp, Hp)
    # GRU2 n hidden side: K = zr2op chunks 3..6 (rows 384..895); h2 at 400..799
    w["Wn2h"] = _pad2(f64["Whn2"].T, Hp, Hp, r0=16)

    w["Wfc2T"] = _pad2(f64["Wfc2"].T, Hp, FFp)    # [512, 640]
    w["Wfc3T"] = _pad2(f64["Wfc3"].T, FFp, FFp)   # [640, 640]
    w["Wfc4T"] = _pad2(f64["Wfc4"].T, FFp, Fp)    # [640, 384]

    weights = {
        k: np.ascontiguousarray(S * v).astype(FP8) for k, v in w.items()
    }

    parts = [
        ("bzr1", _bias_tile(np.concatenate([bx["z"] + f64["bhz1"],
                                            bx["r"] + f64["bhr1"]]), 896)),
        ("bnx1", _bias_tile(S * bx["n"], Hp)),
        ("bnh1", _bias_tile(S * f64["bhn1"], Hp)),
        ("bzr2", _bias_tile(np.concatenate([f64["biz2"] + f64["bhz2"],
                                            f64["bir2"] + f64["bhr2"]]), 896)),
        ("bnx2", _bias_tile(S * f64["bin2"], Hp)),
        ("bnh2", _bias_tile(S * f64["bhn2"], Hp)),
        ("bfc2", _bias_tile(S * f64["bfc2"], FFp)),
        ("bfc3", _bias_tile(S * f64["bfc3"], FFp)),
        ("bfc4", _bias_tile(f64["bfc4"], Fp)),
    ]
    biases = {"biasT": np.concatenate([p[1] for p in parts], axis=1)}
    return weights, biases


def build_nc(nbt=BPC, nb=NB):
    """Build the per-core Bass program. nbt = per-core batch, nb = free-dim tile."""
    nc = bacc.Bacc("TRN2", target_bir_lowering=False, debug=False)
    bf = mybir.dt.bfloat16
    f32 = mybir.dt.float32
    f8 = mybir.dt.float8e4

    xh8 = nc.declare_dram_parameter("xh8", [XHK, nbt], f8, isOutput=False)
    h1T = nc.declare_dram_parameter("h1T", [Hp, nbt], bf, isOutput=False)
    h2T = nc.declare_dram_parameter("h2T", [Hp, nbt], bf, isOutput=False)
    h28 = nc.declare_dram_parameter("h28", [Hp, nbt], f8, isOutput=False)
    wd = {}
    for name, k, m in (
        ("Wzr1", XHK, ZRM), ("Wn1x", Fp, Hp), ("Wn1h", Hp, Hp),
        ("Wzr2", ZR2K, ZRM), ("Wn2x", Hp, Hp), ("Wn2h", Hp, Hp),
        ("Wfc2T", Hp, FFp), ("Wfc3T", FFp, FFp), ("Wfc4T", FFp, Fp),
    ):
        wd[name] = nc.declare_dram_parameter(name, [k, m], f8, isOutput=False)
    biasT_d = nc.declare_dram_parameter("biasT", [128, BIAS_COLS], f32, isOutput=False)
    outT = nc.declare_dram_parameter("outT", [Fp, nbt], bf, isOutput=True)

    n_tiles = nbt // nb
    HC = Hp // 128  # 4 M-chunks per gate
    DR = mybir.MatmulPerfMode.DoubleRow

    with tile.TileContext(nc) as tc:
        with (
            tc.tile_pool(name="wpool", bufs=1) as wpool,
            tc.tile_pool(name="bpool", bufs=1) as bpool,
            tc.tile_pool(name="inp2", bufs=2) as inp2,
            tc.tile_pool(name="inp3", bufs=3) as inp3,
            tc.tile_pool(name="io", bufs=3) as io,
            tc.tile_pool(name="act2", bufs=2) as act2,
            tc.tile_pool(name="act3", bufs=3) as act3,
            tc.tile_pool(name="psum", bufs=2, space="PSUM") as psum,
        ):
            # ACT-table warmup: first ScalarE transcendental carries the
            # ACT_TABLE_LOAD pseudo-inst; keep it off the critical chain.
            warm = bpool.tile([128, 1], f32, tag="warm")
            nc.vector.memset(warm, 0.0)
            nc.scalar.activation(warm, warm, AF.Sigmoid)

            W, BT = {}, {}

            def load_w(name, eng):
                dram = wd[name]
                k, m = dram.shape
                t = wpool.tile([128, k // 128, m], dram.dtype, tag=name)
                r = dram.rearrange("(c p) m -> p c m", p=128)
                for c in range(k // 128):
                    eng.dma_start(out=t[:, c, :], in_=r[:, c, :])
                W[name] = t

            def load_bias():
                biasT = bpool.tile([128, BIAS_COLS], f32, tag="biasT")
                nc.sync.dma_start(out=biasT, in_=biasT_d[:, :])
                for _n, (_o, _c) in BIAS_LAYOUT.items():
                    BT[_n] = biasT[:, _o : _o + _c]

            xh_r = xh8.rearrange("(c p) n -> p c n", p=128)
            h1_bl = h1T.rearrange("(c p) n -> p c n", p=128)
            h2_bl = h2T.rearrange("(c p) n -> p c n", p=128)
            h2_s0 = h28[0:112, :]                     # -> partitions 16..127 of zr2 chunk 3
            h2_s1 = h28[112:496, :].rearrange("(c p) n -> p c n", p=128)
            outT_r = outT.rearrange("(c p) n -> p c n", p=128)

            ST = [dict() for _ in range(n_tiles)]

            def load_inputs(t):
                sl = ts(t, nb)
                xh = inp2.tile([128, 6, nb], f8, tag="xh")      # zr1/nx1/nh1 K operand
                nc.sync.dma_start(out=xh, in_=xh_r[:, :, sl])
                h1s = inp2.tile([128, HC, nb], bf, tag="h1s")   # blend h1
                nc.sync.dma_start(out=h1s, in_=h1_bl[:, :, sl])
                h2s = inp3.tile([128, HC, nb], bf, tag="h2s")   # blend h2
                nc.sync.dma_start(out=h2s, in_=h2_bl[:, :, sl])
                ST[t]["xh"], ST[t]["h1s"], ST[t]["h2s"] = xh, h1s, h2s

            def matseq_dr(ps, Wt, kc, col, mw, rhs_t, r0=0):
                """fp8 accumulation over kc K-chunks using DoubleRow on
                consecutive chunk pairs (odd tail chunk = normal). rhs chunks
                start at r0 within the operand tile."""
                n = (kc + 1) // 2
                for i in range(n):
                    k = 2 * i
                    if k + 2 <= kc:
                        nc.tensor.matmul(
                            ps, Wt[:, k : k + 2, col : col + mw],
                            rhs_t[:, r0 + k : r0 + k + 2, :],
                            start=(i == 0), stop=(i == n - 1), perf_mode=DR)
                    else:
                        nc.tensor.matmul(
                            ps, Wt[:, k, col : col + mw], rhs_t[:, r0 + k, :],
                            start=(i == 0), stop=(i == n - 1))

            def gru(rhs, Wzr, zr_kc, Wnx, nx_kc, Wnh, nh_r0, bzr, bnx, bnh,
                    h_bl, out_full, out_fat3=None, out_sliver=None):
                """One GRU step. rhs: single fp8 K-operand tile; n-gate x side
                reads chunks 0.., hidden side chunks nh_r0.. of rhs. Output h':
                either out_full [128,4,nb] (pad rows compute exact zeros), or
                out_fat3 (chunks 0..2) + out_sliver (chunk 3 rows 0..15) when
                the target's chunk-3 pad lanes must not be written.

                All chain ops run full-lane: pad lanes of every operand are
                exact zeros (zero weights -> zero psum; r_al pad memset), so
                pads propagate as zeros at no extra instruction cost."""
                # z,r preactivations: 7 contiguous M chunks (chunk 6 is 32 wide)
                zro = act3.tile([128, ZRC, nb], bf, tag="zro")
                for c in range(ZRC):
                    mw = min(128, ZRM - c * 128)
                    ps = psum.tile([128, nb], f32, tag="ps_zr")
                    matseq_dr(ps[:mw, :], Wzr, zr_kc, c * 128, mw, rhs)
                    nc.scalar.activation(zro[:mw, c, :], ps[:mw, :], AF.Sigmoid,
                                         bias=bzr[:mw, c : c + 1], scale=1.0 / S)
                # realign r (features at concat rows 400+f) to h's lanes
                r_al = act3.tile([128, HC, nb], bf, tag="r_al")
                # pad lanes of chunk 3 must read as zero; the pool rotates 3
                # physical buffers and the DMA below never touches the pads,
                # so zeroing the first 3 allocations covers every rotation.
                ral_n[0] += 1
                if ral_n[0] <= 3:
                    nc.gpsimd.memset(r_al[:, 3, :], 0.0)
                nc.sync.dma_start(out=r_al[0:112, 0:3, :], in_=zro[16:128, 3:6, :])
                nc.sync.dma_start(out=r_al[112:128, 0:3, :], in_=zro[0:16, 4:7, :])
                nc.sync.dma_start(out=r_al[0:16, 3, :], in_=zro[16:32, 6, :])

                npre = act2.tile([128, HC, nb], f32, tag="npre")
                for m in range(HC):
                    col = m * 128
                    psx = psum.tile([128, nb], f32, tag="ps_nx")
                    matseq_dr(psx, Wnx, nx_kc, col, 128, rhs)
                    psh = psum.tile([128, nb], f32, tag="ps_nh")
                    matseq_dr(psh, Wnh, HC, col, 128, rhs, r0=nh_r0)
                    # rhn = (psh + bnh) * r ; npre = (psx + bnx) + rhn
                    rhn = act3.tile([128, nb], f32, tag="rhn")
                    nc.vector.scalar_tensor_tensor(
                        rhn, psh, bnh[:, m : m + 1],
                        r_al[:, m, :], op0=ALU.add, op1=ALU.mult)
                    nc.vector.scalar_tensor_tensor(
                        npre[:, m, :], psx, bnx[:, m : m + 1],
                        rhn, op0=ALU.add, op1=ALU.add)
                # n = tanh(npre/S), one fat ACT over all 4 chunks
                n_t = act3.tile([128, HC, nb], bf, tag="n_t")
                nc.scalar.activation(n_t, npre, AF.Tanh, scale=1.0 / S)
                # h' = n + z*(h - n); z of chunk 3 rows 16+ is r-junk but meets
                # d==0 there, so pads still come out zero. The subtraction is
                # split across GpSimd and VectorE to balance engine load.
                d = act2.tile([128, HC, nb], bf, tag="d")
                nc.gpsimd.tensor_sub(d[:, 0:2, :], h_bl[:, 0:2, :], n_t[:, 0:2, :])
                nc.vector.tensor_sub(d[:, 2:4, :], h_bl[:, 2:4, :], n_t[:, 2:4, :])
                zd = act2.tile([128, HC, nb], bf, tag="zd")
                nc.vector.tensor_mul(zd, zro[:, 0:HC, :], d)
                if out_full is not None:
                    nc.vector.tensor_add(out_full, n_t, zd)
                else:
                    nc.vector.tensor_add(out_fat3, n_t[:, 0:3, :], zd[:, 0:3, :])
                    nc.vector.tensor_add(out_sliver, n_t[0:16, 3, :], zd[0:16, 3, :])

            def stage_gru1(t):
                sl = ts(t, nb)
                # zr2op = GRU2's [g1|h2] fp8 operand: g1 written by GRU1 blend,
                # h2 stitched in by DMA at concat rows 400..799.
                zr2op = io.tile([128, ZRC, nb], f8, tag="zr2op")
                nc.sync.dma_start(out=zr2op[16:128, 3, :], in_=h2_s0[:, sl])
                nc.sync.dma_start(out=zr2op[:, 4:7, :], in_=h2_s1[:, :, sl])
                ST[t]["zr2op"] = zr2op
                gru(ST[t]["xh"], W["Wzr1"], 6, W["Wn1x"], 3, W["Wn1h"], 2,
                    BT["bzr1"], BT["bnx1"], BT["bnh1"], ST[t]["h1s"],
                    None, zr2op[:, 0:3, :], zr2op[0:16, 3, :])

            def stage_gru2(t):
                zr2op = ST[t]["zr2op"]
                g2 = io.tile([128, HC, nb], f8, tag="g2")
                gru(zr2op, W["Wzr2"], ZRC, W["Wn2x"], HC, W["Wn2h"], 3,
                    BT["bzr2"], BT["bnx2"], BT["bnh2"], ST[t]["h2s"], g2)
                ST[t]["g2"] = g2

            def stage_fc2(t):
                g2 = ST[t].pop("g2")
                f2 = io.tile([128, FFp // 128, nb], f8, tag="f2")
                for m in range(FFp // 128):
                    ps = psum.tile([128, nb], f32, tag="ps_fc")
                    matseq_dr(ps, W["Wfc2T"], HC, m * 128, 128, g2)
                    # f2 = relu(ps + S*b) = S*relu(W g2 + b)
                    nc.vector.tensor_scalar(
                        f2[:, m, :], ps, BT["bfc2"][:, m : m + 1], 0.0,
                        op0=ALU.add, op1=ALU.max)
                ST[t]["f2"] = f2

            def stage_fc3(t):
                f2 = ST[t].pop("f2")
                f3 = io.tile([128, FFp // 128, nb], f8, tag="f3")
                for m in range(FFp // 128):
                    ps = psum.tile([128, nb], f32, tag="ps_fc")
                    matseq_dr(ps, W["Wfc3T"], FFp // 128, m * 128, 128, f2)
                    # f3 = relu(ps/S + S*b) = S*relu(W f2 + b)
                    nc.scalar.activation(f3[:, m, :], ps, AF.Relu,
                                         bias=BT["bfc3"][:, m : m + 1],
                                         scale=1.0 / S)
                ST[t]["f3"] = f3

            def stage_fc4(t):
                sl = ts(t, nb)
                f3 = ST[t].pop("f3")
                o = io.tile([128, Fp // 128, nb], bf, tag="o")
                for m in range(Fp // 128):
                    ps = psum.tile([128, nb], f32, tag="ps_fc")
                    matseq_dr(ps, W["Wfc4T"], FFp // 128, m * 128, 128, f3)
                    nc.scalar.activation(o[:, m, :], ps, AF.Sigmoid,
                                         bias=BT["bfc4"][:, m : m + 1],
                                         scale=1.0 / (S * S))
                nc.sync.dma_start(out=outT_r[:, :, sl], in_=o)

            # startup loads: sync carries GRU1 weights + first inputs; GRU2
            # weights go on the (otherwise idle at start) gpsimd ring; FC
            # weights on the scalar ring (needed only from iteration 2 on).
            load_inputs(0)
            load_w("Wzr1", nc.sync)
            load_bias()
            load_w("Wn1x", nc.sync)
            load_w("Wn1h", nc.sync)
            for name in ("Wzr2", "Wn2x", "Wn2h"):
                load_w(name, nc.gpsimd)

            ral_n = [0]
            PF = 1  # input prefetch depth (iterations ahead)
            for i in range(n_tiles + 4):
                if i < n_tiles:
                    if i + PF < n_tiles:
                        load_inputs(i + PF)
                    stage_gru1(i)
                if i == 0:
                    # FC weights are first needed in iteration 2; issuing the
                    # loads after tile 0's sigmoids keeps the scalar queue free
                    # at startup.
                    for name in ("Wfc2T", "Wfc3T", "Wfc4T"):
                        load_w(name, nc.scalar)
                if 0 <= i - 1 < n_tiles:
                    stage_gru2(i - 1)
                if 0 <= i - 2 < n_tiles:
                    stage_fc2(i - 2)
                if 0 <= i - 3 < n_tiles:
                    stage_fc3(i - 3)
                if 0 <= i - 4 < n_tiles:
                    stage_fc4(i - 4)

    nc.compile()
    return nc


def _shard_inputs(inp, weights, biases):
    x = np.asarray(inp["x"], dtype=np.float32)
    h1 = np.asarray(inp["h1"], dtype=np.float32)
    h2 = np.asarray(inp["h2"], dtype=np.float32)

    xh8 = np.zeros((NCORES, XHK, BPC), dtype=FP8)    # matmul operand [x|h1]
    h1T = np.zeros((NCORES, Hp, BPC), dtype=BF16)    # blend h1
    h2T = np.zeros((NCORES, Hp, BPC), dtype=BF16)    # blend h2
    h28 = np.zeros((NCORES, Hp, BPC), dtype=FP8)     # zr2op h2 stitch source
    for i in range(NCORES):
        sl = slice(i * BPC, (i + 1) * BPC)
        xh8[i, :F] = x[sl].T.astype(FP8)
        xh8[i, F : F + H] = h1[sl].T.astype(FP8)
        h1T[i, :H] = h1[sl].T.astype(BF16)
        h2T[i, :H] = h2[sl].T.astype(BF16)
        h28[i, :H] = h2[sl].T.astype(FP8)

    in_maps = []
    for i in range(NCORES):
        m = {"xh8": xh8[i], "h1T": h1T[i], "h2T": h2T[i], "h28": h28[i]}
        m.update(weights)
        m.update(biases)
        in_maps.append(m)
    return in_maps


def _run(inp, trace=False):
    weights, biases = prepare_weights(inp)
    nc = build_nc()
    in_maps = _shard_inputs(inp, weights, biases)
    res = run_bass_kernel_spmd(nc, in_maps, list(range(NCORES)), trace=trace)
    out = np.empty((B, F), dtype=np.float32)
    for i in range(NCORES):
        out[i * BPC : (i + 1) * BPC] = (
            np.asarray(res.results[i]["outT"][:F]).astype(np.float32).T
        )
    return out, res


def kernel(**inputs) -> np.ndarray:
    out, _ = _run(inputs, trace=False)
    return out


# revision 15
# speedup vs baseline: 1.2603x; 1.0548x over previous
"""NsNet2 single-step (fc1 + 2x GRU cell + 3x FC) Trainium2 kernel.

Strategy:
  - Pure data parallel: batch B=32768 sharded as 4096 rows per NeuronCore (8 cores).
  - Feature-major layout on chip: activations live as [feat, batch]; host
    transposes inputs/outputs (free; off the HW critical path).
  - ALL matmuls fp8(e4m3)+DoubleRow, fp32 PSUM. Weights are scaled by S=16 on
    the host to lift them out of the fp8 denormal range; the scale is divided
    back out for free via ScalarE activation `scale` or the stt bias slot.
  - fc1 folded into GRU1 input-gate weights (fc1 is linear, f1 only feeds GRU1).
  - z,r gates K-concat their input and hidden operands ([x|h1] resp. [g1|h2])
    and M-concat z|r into one 800-col group (7 chunks).
  - The n-gate hidden matmuls reuse the SAME SBUF operand as z,r via shifted
    chunk views (weights re-laid to match), so only one fp8 operand stream is
    loaded per GRU.
  - 5-stage software pipeline over batch tiles (GRU1 | GRU2 | fc2 | fc3 | fc4)
    so the FIFO Tensor queue never head-of-line blocks on the elementwise
    chain of the same tile.
  - Elementwise work spread over ScalarE (sigmoid/tanh/fc3-relu/fc4-sigmoid),
    VectorE (stt chains, fc2-relu, blend mul/add) and GpSimd (blend sub).
"""

import os
import sys

import numpy as np
import ml_dtypes

sys.path.insert(0, "/opt/trn_rl_repo")

import concourse.bacc as bacc
import concourse.bass as bass
import concourse.mybir as mybir
import concourse.tile as tile
from concourse.bass import ts
from concourse.bass_utils import run_bass_kernel_spmd

BF16 = ml_dtypes.bfloat16
FP8 = ml_dtypes.float8_e4m3

B, F, H, FF = 32768, 257, 400, 600
NCORES = 8
BPC = B // NCORES          # 4096 batch rows per core
Hp, FFp, Fp = 512, 640, 384  # padded feature dims
XHK = 768                  # [x(257) | h1(400) | pad(111)] -> 6 zr K chunks
ZR2K = 896                 # [g1(400) | h2(400) | pad(96)] -> 7 chunks
ZRM = 800                  # contiguous [z(400) | r(400)] output cols -> 7 M chunks
ZRC = 7
NB = 512                   # matmul free-dim tile (one PSUM bank of fp32)
S = 16.0                   # fp8 weight scale (denormal avoidance)

AF = mybir.ActivationFunctionType
ALU = mybir.AluOpType

# packed bias column layout: name -> (offset, n_chunks)
BIAS_LAYOUT = {}
_off = 0
for _n, _c in (("bz1", 4), ("br1", 4), ("bnx1", 4), ("bnh1", 4),
               ("bz2", 4), ("br2", 4), ("bnx2", 4), ("bnh2", 4),
               ("bfc2", 5), ("bfc3", 5), ("bfc4", 3)):
    BIAS_LAYOUT[_n] = (_off, _c)
    _off += _c
BIAS_COLS = _off


def _pad2(a, rows, cols, r0=0):
    out = np.zeros((rows, cols), dtype=np.float64)
    out[r0 : r0 + a.shape[0], : a.shape[1]] = a
    return out


def _bias_tile(vec, padded):
    """Pack a [padded] bias vector as [128, padded//128] fp32 (partition-major)."""
    v = np.zeros(padded, dtype=np.float64)
    v[: vec.shape[0]] = vec
    return np.ascontiguousarray(v.reshape(padded // 128, 128).T).astype(np.float32)


def prepare_weights(inp):
    f64 = {k: np.asarray(v, dtype=np.float64) for k, v in inp.items()}
    w = {}

    # fc1 fold for GRU1 input side
    Wx = {}
    bx = {}
    for name in ("z", "r", "n"):
        Wx[name] = (f64[f"Wi{name}1"] @ f64["Wfc1"]).T          # [F, H]
        bx[name] = f64[f"bi{name}1"] + f64[f"Wi{name}1"] @ f64["bfc1"]

    # GRU1 z,r as separate lane-aligned M=512 groups over K=[x(257)|h1(400)]
    for g, name in (("z", "z"), ("r", "r")):
        Wg = np.zeros((XHK, Hp), dtype=np.float64)
        Wg[:F, :H] = Wx[name]
        Wg[F : F + H, :H] = f64[f"Wh{name}1"].T
        w[f"W{g}1"] = Wg
    # GRU1 n input side: K = xh chunks 0..2 (rows 0..383; rows 257+ are h1 -> 0)
    w["Wn1x"] = _pad2(Wx["n"], Fp, Hp)
    # GRU1 n hidden side: K = xh chunks 2..5 (rows 256..767); h1 lives at 257..656
    w["Wn1h"] = _pad2(f64["Whn1"].T, Hp, Hp, r0=1)

    # GRU2 z,r over K=[g1(400) | h2(400)] (zr2op layout, 896 rows)
    for g in ("z", "r"):
        Wg = np.zeros((ZR2K, Hp), dtype=np.float64)
        Wg[:H, :H] = f64[f"Wi{g}2"].T
        Wg[H : 2 * H, :H] = f64[f"Wh{g}2"].T
        w[f"W{g}2"] = Wg
    # GRU2 n input side: K = zr2op chunks 0..3 (rows 0..511; g1 at 0..399)
    w["Wn2x"] = _pad2(f64["Win2"].T, Hp, Hp)
    # GRU2 n hidden side: K = zr2op chunks 3..6 (rows 384..895); h2 at 400..799
    w["Wn2h"] = _pad2(f64["Whn2"].T, Hp, Hp, r0=16)

    w["Wfc2T"] = _pad2(f64["Wfc2"].T, Hp, FFp)    # [512, 640]
    w["Wfc3T"] = _pad2(f64["Wfc3"].T, FFp, FFp)   # [640, 640]
    w["Wfc4T"] = _pad2(f64["Wfc4"].T, FFp, Fp)    # [640, 384]

    weights = {
        k: np.ascontiguousarray(S * v).astype(FP8) for k, v in w.items()
    }

    parts = [
        ("bz1", _bias_tile(bx["z"] + f64["bhz1"], Hp)),
        ("br1", _bias_tile(bx["r"] + f64["bhr1"], Hp)),
        ("bnx1", _bias_tile(S * bx["n"], Hp)),
        ("bnh1", _bias_tile(S * f64["bhn1"], Hp)),
        ("bz2", _bias_tile(f64["biz2"] + f64["bhz2"], Hp)),
        ("br2", _bias_tile(f64["bir2"] + f64["bhr2"], Hp)),
        ("bnx2", _bias_tile(S * f64["bin2"], Hp)),
        ("bnh2", _bias_tile(S * f64["bhn2"], Hp)),
        ("bfc2", _bias_tile(S * f64["bfc2"], FFp)),
        ("bfc3", _bias_tile(S * f64["bfc3"], FFp)),
        ("bfc4", _bias_tile(f64["bfc4"], Fp)),
    ]
    biases = {"biasT": np.concatenate([p[1] for p in parts], axis=1)}
    return weights, biases


def build_nc(nbt=BPC, nb=NB):
    """Build the per-core Bass program. nbt = per-core batch, nb = free-dim tile."""
    nc = bacc.Bacc("TRN2", target_bir_lowering=False, debug=False)
    bf = mybir.dt.bfloat16
    f32 = mybir.dt.float32
    f8 = mybir.dt.float8e4

    xh8 = nc.declare_dram_parameter("xh8", [XHK, nbt], f8, isOutput=False)
    h1T = nc.declare_dram_parameter("h1T", [Hp, nbt], bf, isOutput=False)
    h2T = nc.declare_dram_parameter("h2T", [Hp, nbt], bf, isOutput=False)
    h28 = nc.declare_dram_parameter("h28", [Hp, nbt], f8, isOutput=False)
    wd = {}
    for name, k, m in (
        ("Wz1", XHK, Hp), ("Wr1", XHK, Hp), ("Wn1x", Fp, Hp), ("Wn1h", Hp, Hp),
        ("Wz2", ZR2K, Hp), ("Wr2", ZR2K, Hp), ("Wn2x", Hp, Hp), ("Wn2h", Hp, Hp),
        ("Wfc2T", Hp, FFp), ("Wfc3T", FFp, FFp), ("Wfc4T", FFp, Fp),
    ):
        wd[name] = nc.declare_dram_parameter(name, [k, m], f8, isOutput=False)
    biasT_d = nc.declare_dram_parameter("biasT", [128, BIAS_COLS], f32, isOutput=False)
    outT = nc.declare_dram_parameter("outT", [Fp, nbt], bf, isOutput=True)

    n_tiles = nbt // nb
    HC = Hp // 128  # 4 M-chunks per gate
    DR = mybir.MatmulPerfMode.DoubleRow

    with tile.TileContext(nc) as tc:
        with (
            tc.tile_pool(name="wpool", bufs=1) as wpool,
            tc.tile_pool(name="bpool", bufs=1) as bpool,
            tc.tile_pool(name="inp2", bufs=2) as inp2,
            tc.tile_pool(name="inp3", bufs=3) as inp3,
            tc.tile_pool(name="io", bufs=3) as io,
            tc.tile_pool(name="act2", bufs=2) as act2,
            tc.tile_pool(name="act3", bufs=3) as act3,
            tc.tile_pool(name="psum", bufs=2, space="PSUM") as psum,
        ):
            # ACT-table warmup: first ScalarE transcendental carries the
            # ACT_TABLE_LOAD pseudo-inst; keep it off the critical chain.
            warm = bpool.tile([128, 1], f32, tag="warm")
            nc.vector.memset(warm, 0.0)
            nc.scalar.activation(warm, warm, AF.Sigmoid)

            W, BT = {}, {}

            def load_w(name, eng):
                dram = wd[name]
                k, m = dram.shape
                t = wpool.tile([128, k // 128, m], dram.dtype, tag=name)
                r = dram.rearrange("(c p) m -> p c m", p=128)
                for c in range(k // 128):
                    eng.dma_start(out=t[:, c, :], in_=r[:, c, :])
                W[name] = t

            def load_bias():
                biasT = bpool.tile([128, BIAS_COLS], f32, tag="biasT")
                nc.sync.dma_start(out=biasT, in_=biasT_d[:, :])
                for _n, (_o, _c) in BIAS_LAYOUT.items():
                    BT[_n] = biasT[:, _o : _o + _c]

            xh_r = xh8.rearrange("(c p) n -> p c n", p=128)
            h1_bl = h1T.rearrange("(c p) n -> p c n", p=128)
            h2_bl = h2T.rearrange("(c p) n -> p c n", p=128)
            h2_s0 = h28[0:112, :]                     # -> partitions 16..127 of zr2 chunk 3
            h2_s1 = h28[112:496, :].rearrange("(c p) n -> p c n", p=128)
            outT_r = outT.rearrange("(c p) n -> p c n", p=128)

            ST = [dict() for _ in range(n_tiles)]

            def load_inputs(t):
                sl = ts(t, nb)
                xh = inp2.tile([128, 6, nb], f8, tag="xh")      # zr1/nx1/nh1 K operand
                nc.sync.dma_start(out=xh, in_=xh_r[:, :, sl])
                h1s = inp2.tile([128, HC, nb], bf, tag="h1s")   # blend h1
                nc.sync.dma_start(out=h1s, in_=h1_bl[:, :, sl])
                h2s = inp3.tile([128, HC, nb], bf, tag="h2s")   # blend h2
                nc.sync.dma_start(out=h2s, in_=h2_bl[:, :, sl])
                ST[t]["xh"], ST[t]["h1s"], ST[t]["h2s"] = xh, h1s, h2s

            def matseq_dr(ps, Wt, kc, col, mw, rhs_t, r0=0):
                """fp8 accumulation over kc K-chunks using DoubleRow on
                consecutive chunk pairs (odd tail chunk = normal). rhs chunks
                start at r0 within the operand tile."""
                n = (kc + 1) // 2
                for i in range(n):
                    k = 2 * i
                    if k + 2 <= kc:
                        nc.tensor.matmul(
                            ps, Wt[:, k : k + 2, col : col + mw],
                            rhs_t[:, r0 + k : r0 + k + 2, :],
                            start=(i == 0), stop=(i == n - 1), perf_mode=DR)
                    else:
                        nc.tensor.matmul(
                            ps, Wt[:, k, col : col + mw], rhs_t[:, r0 + k, :],
                            start=(i == 0), stop=(i == n - 1))

            def gru_mm(rhs, Wz, Wr, zr_kc, Wnx, nx_kc, Wnh, nh_r0,
                       bz, br, bnx, bnh):
                """Matmul + activation-chain part of one GRU step. Groups are
                interleaved per feature chunk m ([Z R NX NH] x 4) so each PSUM
                tag is reused only every 4th group and consumers keep up.
                Returns (z_t, n_t); the h' blend is issued separately so the
                VectorE queue is not head-of-line blocked on late tanh results.

                All chain ops run full-lane: pad lanes of every operand are
                exact zeros (zero weights -> zero psum, zero bias), except
                z/r pads which are sigmoid(0)=0.5 and get multiplied by the
                zero pads of the other blend operand."""
                z_t = act3.tile([128, HC, nb], bf, tag="z_t")
                r_t = act3.tile([128, HC, nb], bf, tag="r_t")
                npre = act2.tile([128, HC, nb], f32, tag="npre")
                for m in range(HC):
                    col = m * 128
                    psz = psum.tile([128, nb], f32, tag="ps_zr")
                    matseq_dr(psz, Wz, zr_kc, col, 128, rhs)
                    nc.scalar.activation(z_t[:, m, :], psz, AF.Sigmoid,
                                         bias=bz[:, m : m + 1], scale=1.0 / S)
                    psr = psum.tile([128, nb], f32, tag="ps_zr")
                    matseq_dr(psr, Wr, zr_kc, col, 128, rhs)
                    nc.scalar.activation(r_t[:, m, :], psr, AF.Sigmoid,
                                         bias=br[:, m : m + 1], scale=1.0 / S)
                    psx = psum.tile([128, nb], f32, tag="ps_nx")
                    matseq_dr(psx, Wnx, nx_kc, col, 128, rhs)
                    psh = psum.tile([128, nb], f32, tag="ps_nh")
                    matseq_dr(psh, Wnh, HC, col, 128, rhs, r0=nh_r0)
                    # rhn = (psh + bnh) * r ; npre = (psx + bnx) + rhn
                    rhn = act3.tile([128, nb], f32, tag="rhn")
                    nc.vector.scalar_tensor_tensor(
                        rhn, psh, bnh[:, m : m + 1],
                        r_t[:, m, :], op0=ALU.add, op1=ALU.mult)
                    nc.vector.scalar_tensor_tensor(
                        npre[:, m, :], psx, bnx[:, m : m + 1],
                        rhn, op0=ALU.add, op1=ALU.add)
                # n = tanh(npre/S), one fat ACT over all 4 chunks
                n_t = act3.tile([128, HC, nb], bf, tag="n_t")
                nc.scalar.activation(n_t, npre, AF.Tanh, scale=1.0 / S)
                return z_t, n_t

            def blend(z_t, n_t, h_bl, out_full, out_fat3=None, out_sliver=None):
                """h' = n + z*(h - n). out_full [128,4,nb] when the target's
                pad lanes may be written (they compute to zero); fat3 + sliver
                otherwise. The subtraction is split across GpSimd and VectorE
                to balance engine load."""
                d = act2.tile([128, HC, nb], bf, tag="d")
                nc.gpsimd.tensor_sub(d[:, 0:2, :], h_bl[:, 0:2, :], n_t[:, 0:2, :])
                nc.vector.tensor_sub(d[:, 2:4, :], h_bl[:, 2:4, :], n_t[:, 2:4, :])
                zd = act2.tile([128, HC, nb], bf, tag="zd")
                nc.vector.tensor_mul(zd, z_t, d)
                if out_full is not None:
                    nc.vector.tensor_add(out_full, n_t, zd)
                else:
                    nc.vector.tensor_add(out_fat3, n_t[:, 0:3, :], zd[:, 0:3, :])
                    nc.vector.tensor_add(out_sliver, n_t[0:16, 3, :], zd[0:16, 3, :])

            def stage_gru1(t):
                sl = ts(t, nb)
                # zr2op = GRU2's [g1|h2] fp8 operand: g1 written by GRU1 blend,
                # h2 stitched in by DMA at concat rows 400..799.
                zr2op = io.tile([128, ZRC, nb], f8, tag="zr2op")
                nc.sync.dma_start(out=zr2op[16:128, 3, :], in_=h2_s0[:, sl])
                nc.sync.dma_start(out=zr2op[:, 4:7, :], in_=h2_s1[:, :, sl])
                ST[t]["zr2op"] = zr2op
                z_t, n_t = gru_mm(ST[t]["xh"], W["Wz1"], W["Wr1"], 6,
                                  W["Wn1x"], 3, W["Wn1h"], 2,
                                  BT["bz1"], BT["br1"], BT["bnx1"], BT["bnh1"])
                return (z_t, n_t, ST[t]["h1s"],
                        None, zr2op[:, 0:3, :], zr2op[0:16, 3, :])

            def stage_gru2(t):
                zr2op = ST[t]["zr2op"]
                g2 = io.tile([128, HC, nb], f8, tag="g2")
                z_t, n_t = gru_mm(zr2op, W["Wz2"], W["Wr2"], ZRC,
                                  W["Wn2x"], HC, W["Wn2h"], 3,
                                  BT["bz2"], BT["br2"], BT["bnx2"], BT["bnh2"])
                ST[t]["g2"] = g2
                return (z_t, n_t, ST[t]["h2s"], g2, None, None)

            def stage_fc2(t):
                g2 = ST[t].pop("g2")
                f2 = io.tile([128, FFp // 128, nb], f8, tag="f2")
                for m in range(FFp // 128):
                    ps = psum.tile([128, nb], f32, tag="ps_fc")
                    matseq_dr(ps, W["Wfc2T"], HC, m * 128, 128, g2)
                    # f2 = relu(ps + S*b) = S*relu(W g2 + b)
                    nc.vector.tensor_scalar(
                        f2[:, m, :], ps, BT["bfc2"][:, m : m + 1], 0.0,
                        op0=ALU.add, op1=ALU.max)
                ST[t]["f2"] = f2

            def stage_fc3(t):
                f2 = ST[t].pop("f2")
                f3 = io.tile([128, FFp // 128, nb], f8, tag="f3")
                for m in range(FFp // 128):
                    ps = psum.tile([128, nb], f32, tag="ps_fc")
                    matseq_dr(ps, W["Wfc3T"], FFp // 128, m * 128, 128, f2)
                    # f3 = relu(ps/S + S*b) = S*relu(W f2 + b)
                    nc.scalar.activation(f3[:, m, :], ps, AF.Relu,
                                         bias=BT["bfc3"][:, m : m + 1],
                                         scale=1.0 / S)
                ST[t]["f3"] = f3

            def stage_fc4(t):
                sl = ts(t, nb)
                f3 = ST[t].pop("f3")
                o = io.tile([128, Fp // 128, nb], bf, tag="o")
                for m in range(Fp // 128):
                    ps = psum.tile([128, nb], f32, tag="ps_fc")
                    matseq_dr(ps, W["Wfc4T"], FFp // 128, m * 128, 128, f3)
                    nc.scalar.activation(o[:, m, :], ps, AF.Sigmoid,
                                         bias=BT["bfc4"][:, m : m + 1],
                                         scale=1.0 / (S * S))
                nc.sync.dma_start(out=outT_r[:, :, sl], in_=o)

            # startup loads: sync carries GRU1 weights + first inputs; GRU2
            # weights go on the (otherwise idle at start) gpsimd ring; FC
            # weights on the scalar ring (needed only from iteration 2 on).
            # Startup: only what iteration 0 needs loads immediately; GRU2 and
            # FC weights are pushed past the critical window so they do not
            # steal HBM bandwidth from xh/Wz1/Wr1 (first-matmul gating).
            load_inputs(0)
            load_w("Wz1", nc.sync)
            load_bias()
            load_w("Wr1", nc.sync)
            load_w("Wn1x", nc.sync)
            load_w("Wn1h", nc.sync)
            with tc.tile_wait_until(0.004):
                for name in ("Wz2", "Wr2", "Wn2x", "Wn2h"):
                    load_w(name, nc.gpsimd)
            with tc.tile_wait_until(0.009):
                for name in ("Wfc2T", "Wfc3T", "Wfc4T"):
                    load_w(name, nc.scalar)

            PF = 1  # input prefetch depth (iterations ahead)
            for i in range(n_tiles + 4):
                blends = []
                if i < n_tiles:
                    if i + PF < n_tiles:
                        load_inputs(i + PF)
                    blends.append(stage_gru1(i))
                if 0 <= i - 1 < n_tiles:
                    blends.append(stage_gru2(i - 1))
                if 0 <= i - 2 < n_tiles:
                    stage_fc2(i - 2)
                if 0 <= i - 3 < n_tiles:
                    stage_fc3(i - 3)
                if 0 <= i - 4 < n_tiles:
                    stage_fc4(i - 4)
                # blends last: their inputs (tanh) land late in the iteration,
                # and anything queued behind them would head-of-line block.
                for b in blends:
                    blend(*b)

    nc.compile()
    return nc


def _shard_inputs(inp, weights, biases):
    x = np.asarray(inp["x"], dtype=np.float32)
    h1 = np.asarray(inp["h1"], dtype=np.float32)
    h2 = np.asarray(inp["h2"], dtype=np.float32)

    xh8 = np.zeros((NCORES, XHK, BPC), dtype=FP8)    # matmul operand [x|h1]
    h1T = np.zeros((NCORES, Hp, BPC), dtype=BF16)    # blend h1
    h2T = np.zeros((NCORES, Hp, BPC), dtype=BF16)    # blend h2
    h28 = np.zeros((NCORES, Hp, BPC), dtype=FP8)     # zr2op h2 stitch source
    for i in range(NCORES):
        sl = slice(i * BPC, (i + 1) * BPC)
        xh8[i, :F] = x[sl].T.astype(FP8)
        xh8[i, F : F + H] = h1[sl].T.astype(FP8)
        h1T[i, :H] = h1[sl].T.astype(BF16)
        h2T[i, :H] = h2[sl].T.astype(BF16)
        h28[i, :H] = h2[sl].T.astype(FP8)

    in_maps = []
    for i in range(NCORES):
        m = {"xh8": xh8[i], "h1T": h1T[i], "h2T": h2T[i], "h28": h28[i]}
        m.update(weights)
        m.update(biases)
        in_maps.append(m)
    return in_maps


def _run(inp, trace=False):
    weights, biases = prepare_weights(inp)
    nc = build_nc()
    in_maps = _shard_inputs(inp, weights, biases)
    res = run_bass_kernel_spmd(nc, in_maps, list(range(NCORES)), trace=trace)
    out = np.empty((B, F), dtype=np.float32)
    for i in range(NCORES):
        out[i * BPC : (i + 1) * BPC] = (
            np.asarray(res.results[i]["outT"][:F]).astype(np.float32).T
        )
    return out, res


def kernel(**inputs) -> np.ndarray:
    out, _ = _run(inputs, trace=False)
    return out
